# revision 9
# baseline (speedup 1.0000x reference)
"""LocalContrastEnhancement (15x15 box filter mean/var normalization) on 8 trn2 cores.

out = (x - mean) / (sqrt(max(var, 1e-6)) + 1e-6)
mean = box15(x)/225, var = box15(x^2)/225 - mean^2, zero-padded box filter.

Sharding: pure data parallel, 1 image (3,1024,1024) per NeuronCore.

Per-core algorithm (exact at borders via the "pad = -0.5" centering trick):
  x' = x - 0.5 with padding treated as value -0.5 (raw pad zeros in the
  x buffer; the squares stream computes (0-0.5)^2 = 0.25 for pads inline).
  Horizontal 15-window sums are computed by CUSTOM DVE scan ops (the stock
  tensor_tensor_scan routes the scan feedback through out_a and runs at
  half throughput, ~2.2 cyc/elem; a custom Spec scan() lowers to the
  one-cycle CURR_ALU_OUT recurrence, ~1.14 cyc/elem — measured 1229ns vs
  2298ns per [<=128, 1031] scan):
    o1[t] = init0 + sum_{i<=t} (x[i] - x[i-15])          (init0 = 0)
    o2[t] = 3.75 + sum_{i<=t} ((x[i]-.5)^2 - (x[i-15]-.5)^2)
  o1 carries a +7.5 shift vs the centered window sum (init0 = 0 instead of
  -7.5).  After the vertical band matmul with -1 taps this makes
    PD = -(band x o1) = -S1_true     exactly, for ALL stripe variants
  (each band column m has taps(m) = 15 - n(m) taps; the 7.5*taps(m) shift
  equals the old per-row d_scal correction).  So:
    s1sq = (PD + 112.5)^2 = (S1_true - 112.5)^2 = S1~^2   (uniform bias!)
    PD += 225x (fp16 identity matmul)  ->  PD = 225(x - mean)
    P2 = 225*S2~ - s1sq (via -I matmul), rsqrt bias = 843.75*n(m)
    out = PD * rsqrt(P2 + bias)        (plain tensor_tensor mult)

Engine mapping (v3):
  - horizontal 15-box sums: 2 custom DVE scans straight off the raw x
    buffer (the squaring is fused into the scan body: sq(Src-0.5));
  - vertical 15-box: PE band matmuls (fp16, free-dim bound);
  - ACT (one table set): x->fp16 copy, s1sq square, rsqrt;
  - DVE: 2 custom scans + final PD*R tensor_tensor.
"""

import numpy as np
import ml_dtypes

C, H, W = 3, 1024, 1024
NCORES = 8
KS = 15
HALF = 7  # kernel_size // 2
PADL = 15  # left zero pad cols in the row buffer
PADR = 7  # right zero pad cols
BW = PADL + W + PADR  # 1046 row buffer width
SCAN_N = W + HALF  # 1031 scan output length (first 7 are t<0 positions)
MSTR = 114  # interior out-stripe height (128 - 14 halo)
NHALF = 512  # matmul moving free size (one PSUM bank of f32)

_CACHE = {}


def _get_custom_ops():
    """Register (once) and return the two custom DVE scan ops."""
    if "ops" in _CACHE:
        return _CACHE["ops"]
    import concourse.dve_ops as D
    from concourse.dve_uop import DveOpSpec
    from concourse.dve_spec import (
        Spec, Src0, Src1, C0, C1, AluOp, scan, sq, lower, _has_src1,
    )

    def make_op(name, spec):
        for o in D.OPS:
            if o.name == name:
                return o
        row = D._CUSTOM_DVE_ROW_BASE + len(D.OPS)
        shas = {}
        for ver in ("v3", "v4"):
            s = DveOpSpec(
                name=name, opcode=row, uops=lower(spec, ver=ver),
                rd1_en=_has_src1(spec),
            )
            shas[ver] = s.sha(ver)
        op = D.DveOp(name, spec, subdim=False, uops_sha=shas)
        D.OPS.append(op)
        D._SUB_OPCODE_FOR_NAME[name] = row
        D.CUSTOM_DVE_SPECS[name] = spec
        return op

    box = make_op(
        "LCE_BOX_SCAN",
        Spec(
            body=scan(AluOp.ADD, Src0 - Src1, init=C0),
            reference=lambda in0, in1, s0, s1, imm2: s0
            + np.cumsum(in0 - in1, axis=-1),
        ),
    )
    from concourse.dve_spec import C2

    boxsq = make_op(
        "LCE_BOXSQ_SCAN_SC",
        Spec(
            body=scan(AluOp.ADD, sq(Src0 - C1) - sq(Src1 - C1), init=C0) * C2,
            reference=lambda in0, in1, s0, s1, imm2: (
                s0 + np.cumsum((in0 - s1) ** 2 - (in1 - s1) ** 2, axis=-1)
            )
            * imm2,
        ),
    )
    _CACHE["ops"] = (box, boxsq)
    return _CACHE["ops"]


def _stripes():
    """(r_in0, K, r_out0, M, k_ofs) per stripe; k_ofs=7 marks the top stripe
    (its band/id constants are the mid ones shifted up 7 rows)."""
    out = []
    r_out = 0
    while r_out < H:
        m = min(MSTR, H - r_out)
        r_in0 = max(r_out - HALF, 0)
        r_in1 = min(r_out + m - 1 + HALF, H - 1)
        k = r_in1 - r_in0 + 1
        k_ofs = HALF - (r_out - r_in0)
        out.append((r_in0, k, r_out, m, k_ofs))
        r_out += m
    return out


def _const_mats():
    band = np.zeros((128, MSTR), dtype=np.float32)
    iden = np.zeros((128, MSTR), dtype=np.float32)
    for m in range(MSTR):
        band[m : m + KS, m] = 1.0
        iden[m + HALF, m] = 225.0
    band_top = np.zeros_like(band)
    band_top[0:121, :] = band[7:128, :]
    iden_top = np.zeros_like(iden)
    iden_top[0:121, :] = iden[7:128, :]
    # negI for the var fold: out row m subtracts s1sq row m (same partition)
    negi = np.zeros((128, MSTR), dtype=np.float32)
    for m in range(MSTR):
        negi[m, m] = -1.0
    # One shared stationary (-band) serves BOTH the o1 (mean) and the
    # -225-scaled o2 (squares) phase-1 matmuls.
    bands = np.stack([-band, -band_top, negi], axis=1)  # [128, 3, 114] fp16
    idens = np.stack([iden, iden_top], axis=1).astype(np.float16)  # [128, 2, 114]

    # Per-out-row rsqrt bias: vertical windows past the image miss n all-pad
    # rows, each contributing 3.75 to S2~ (so 843.75*n to 225*S2~).  Column
    # 3 holds the uniform +112.5 bias for the s1sq square.
    m_idx = np.arange(128)
    n_top = np.maximum(0, HALF - m_idx).astype(np.float32)
    n_bot = np.maximum(0, m_idx - 104).astype(np.float32)  # bottom stripe M=112
    corr = np.zeros((128, 4), dtype=np.float32)
    corr[:, 0] = 843.75 * n_top
    corr[:, 1] = 843.75 * n_bot
    corr[:, 2] = 0.0  # interior
    corr[:, 3] = 112.5
    return bands.astype(np.float16), idens, corr


def _build_nc():
    import concourse.bass as bass
    import concourse.bacc as bacc
    import concourse.tile as tile
    from concourse import mybir
    import bass_rust as _bass_rust
    from concourse.hw_specs import get_activation_tables

    f32 = mybir.dt.float32
    fp16 = mybir.dt.float16
    Alu = mybir.AluOpType
    Act = mybir.ActivationFunctionType

    BOX, BOXSQ = _get_custom_ops()

    class _LceBacc(bacc.Bacc):
        """Bacc with act-table selection pinned to the one set that holds
        Square+Copy+Abs_reciprocal_sqrt (the default chooser thrashes)."""

        def insert_act_table_loads(self):
            tables = [
                (name, funcs if name == "abs_reciprocal_sqrt_and_small" else set())
                for name, funcs in get_activation_tables(self.m.arch).items()
            ]
            _bass_rust.insert_act_table_loads(self, tables)

    nc = _LceBacc(trn_type="TRN2", target_bir_lowering=False)
    x_d = nc.dram_tensor("x", [C, H, W], f32, kind="ExternalInput")
    bands_d = nc.dram_tensor("bands", [128, 3, MSTR], fp16, kind="ExternalInput")
    iden_d = nc.dram_tensor("iden", [128, 2, MSTR], fp16, kind="ExternalInput")
    corr_d = nc.dram_tensor("corr", [128, 4], f32, kind="ExternalInput")
    y_d = nc.dram_tensor("y", [C, H, W], f32, kind="ExternalOutput")

    stripes = _stripes()

    from contextlib import ExitStack

    with tile.TileContext(nc) as tc, ExitStack() as ctx:
        singles = ctx.enter_context(tc.tile_pool(name="singles", bufs=1))
        io_pool = ctx.enter_context(tc.tile_pool(name="io", bufs=1))
        s1sq_p = ctx.enter_context(tc.tile_pool(name="s1sq", bufs=4))
        r_p = ctx.enter_context(tc.tile_pool(name="rts", bufs=3))
        out_p = ctx.enter_context(tc.tile_pool(name="outb", bufs=4))
        ps_p = ctx.enter_context(tc.tile_pool(name="ps", bufs=2, space="PSUM"))

        bands_t = singles.tile([128, 3, MSTR], fp16)
        iden_t = singles.tile([128, 2, MSTR], fp16)
        corr_t = singles.tile([128, 4], f32)
        nc.sync.dma_start(out=bands_t[:, :, :], in_=bands_d[:, :, :])
        nc.sync.dma_start(out=iden_t[:, :, :], in_=iden_d[:, :, :])
        nc.sync.dma_start(out=corr_t[:, :], in_=corr_d[:, :])

        NBUF = 4
        xb = [io_pool.tile([128, BW], f32, tag=f"xb{i}", name=f"xb{i}") for i in range(NBUF)]
        ob1 = [io_pool.tile([128, SCAN_N], fp16, tag=f"ob1{i}", name=f"ob1{i}") for i in range(NBUF)]
        ob2 = [io_pool.tile([128, SCAN_N], fp16, tag=f"ob2{i}", name=f"ob2{i}") for i in range(NBUF)]
        x16 = [io_pool.tile([128, W], fp16, tag=f"x16{i}", name=f"x16{i}") for i in range(NBUF)]
        for i in range(NBUF):
            nc.vector.memset(xb[i][:, 0:PADL], 0.0)
            nc.vector.memset(xb[i][:, PADL + W : BW], 0.0)

        # ACT hardware instructions carry at most ONE sync wait; warm-up
        # activations make ACT observe the const-DMA queues and DVE memset
        # ticks here so loop activations don't accumulate extra waits.
        warm1 = singles.tile([128, 1], f32)
        warm2 = singles.tile([128, 1], f32)
        warm3 = singles.tile([128, 1], f32)
        warm4 = singles.tile([128, 1], f32)
        nc.scalar.activation(out=warm1[:, :], in_=corr_t[:, 0:1], func=Act.Square)
        nc.scalar.activation(out=warm2[:, :], in_=iden_t[:, 0, 0:1], func=Act.Square)
        nc.scalar.activation(out=warm3[:, :], in_=xb[0][:, 0:1], func=Act.Square)
        nc.scalar.activation(
            out=warm4[:, :], in_=warm3[:, :], func=Act.Abs_reciprocal_sqrt
        )

        it = 0
        for c in range(C):
            for r_in0, K, r_out0, M, k_ofs in stripes:
                i3 = it % NBUF
                it += 1
                xt, o1, o2, xh = xb[i3], ob1[i3], ob2[i3], x16[i3]

                nc.sync.dma_start(
                    out=xt[0:K, PADL : PADL + W],
                    in_=x_d[c, r_in0 : r_in0 + K, :],
                )

                # fp16 copy of x for the (fast) fp16 identity matmul —
                # on the otherwise-idle GpSimd engine
                nc.gpsimd.tensor_copy(
                    out=xh[0:K, :],
                    in_=xt[0:K, PADL : PADL + W],
                )

                # Horizontal sliding 15-sums via the custom DVE scans.
                # o1[t] = 0 + sum_{i<=t} (x[i] - x[i-15])   (pads contribute
                # the exact centering; +7.5-shifted output = the d-fold)
                nc.vector._custom_dve(
                    BOX,
                    out=o1[0:K, 0:SCAN_N],
                    in0=xt[0:K, PADL : PADL + SCAN_N],
                    in1=xt[0:K, 0:SCAN_N],
                    s0=0.0,
                )
                # o2[t] = -225 * (3.75 + sum ((x[i]-.5)^2 - (x[i-15]-.5)^2))
                # (the -225 scale makes the shared -band stationary produce
                # +225*S2~ in the P2 half of the PSUM tile)
                nc.vector._custom_dve(
                    BOXSQ,
                    out=o2[0:K, 0:SCAN_N],
                    in0=xt[0:K, PADL : PADL + SCAN_N],
                    in1=xt[0:K, 0:SCAN_N],
                    s0=3.75,
                    s1=0.5,
                    imm2=-225.0,
                )

                bsel = 1 if k_ofs else 0  # top-stripe band constants at +1
                isel = 1 if k_ofs else 0
                vv = 0 if k_ofs else (1 if r_out0 + M == H else 2)
                p2_bias = corr_t[0:M, vv : vv + 1]
                sq_bias = corr_t[0:M, 3:4]  # +112.5, uniform

                # one 4-bank PSUM tile: cols [0,W) = PD, cols [W,2W) = P2
                pp = ps_p.tile([MSTR, 2 * W], f32)
                # phase 1 (one shared stationary, 4 moving blocks):
                #   PD = -S1_true ; P2 = 225*S2~
                for j0, src in (
                    (0, o1),
                    (NHALF, o1),
                    (W, o2),
                    (W + NHALF, o2),
                ):
                    nc.tensor.matmul(
                        pp[0:M, j0 : j0 + NHALF],
                        bands_t[0:K, bsel, 0:M],
                        src[0:K, HALF + (j0 % W) : HALF + (j0 % W) + NHALF],
                        start=True,
                        stop=False,
                    )
                # s1sq = (S1_true - 112.5)^2 = (PD + 112.5)^2, fp16
                s1sq = s1sq_p.tile([MSTR, W], fp16)
                nc.scalar.activation(
                    out=s1sq[0:M, :],
                    in_=pp[0:M, 0:W],
                    func=Act.Square,
                    bias=sq_bias,
                )
                # phase 2: PD += 225x ; P2 -= s1sq
                for j0 in (0, NHALF):
                    nc.tensor.matmul(
                        pp[0:M, j0 : j0 + NHALF],
                        iden_t[0:K, isel, 0:M],
                        xh[0:K, j0 : j0 + NHALF],
                        start=False,
                        stop=True,
                        skip_group_check=True,
                    )
                for j0 in (0, NHALF):
                    nc.tensor.matmul(
                        pp[0:M, W + j0 : W + j0 + NHALF],
                        bands_t[0:M, 2, 0:M],
                        s1sq[0:M, j0 : j0 + NHALF],
                        start=False,
                        stop=True,
                    )
                # R = rsqrt(var') in one ACT op (probed: 4.4e-5 max rel
                # err); var' = P2 + corr folded into the activation bias.
                rts = r_p.tile([MSTR, W], f32)
                nc.scalar.activation(
                    out=rts[0:M, :],
                    in_=pp[0:M, W : 2 * W],
                    func=Act.Abs_reciprocal_sqrt,
                    bias=p2_bias,
                )
                # out = PD * R
                outb = out_p.tile([MSTR, W], f32)
                nc.vector.tensor_tensor(
                    out=outb[0:M, :],
                    in0=pp[0:M, 0:W],
                    in1=rts[0:M, :],
                    op=Alu.mult,
                )
                nc.sync.dma_start(
                    out=y_d[c, r_out0 : r_out0 + M, :], in_=outb[0:M, :]
                )

    nc.finalize()
    return nc


def _get_nc():
    if "nc" not in _CACHE:
        _CACHE["nc"] = _build_nc()
    return _CACHE["nc"]


def kernel(x: np.ndarray, _trace: bool = False, _tmpdir=None) -> np.ndarray:
    from concourse.bass_utils import run_bass_kernel_spmd

    assert x.shape == (NCORES, C, H, W), x.shape
    nc = _get_nc()
    bands, iden, corr = _const_mats()
    in_maps = [
        {
            "x": np.ascontiguousarray(x[i]).astype(np.float32, copy=False),
            "bands": bands,
            "iden": iden,
            "corr": corr,
        }
        for i in range(NCORES)
    ]
    res = run_bass_kernel_spmd(
        nc,
        in_maps,
        core_ids=list(range(NCORES)),
        trace=_trace,
        tmpdir=_tmpdir,
    )
    _CACHE["last_results"] = res
    out = np.stack([r["y"] for r in res.results], axis=0)
    return out


if __name__ == "__main__":
    rng = np.random.default_rng(0)
    x = rng.random((NCORES, C, H, W), dtype=np.float32)
    y = kernel(x)
    print(y.shape, y.dtype, float(np.abs(y).mean()))


# revision 10
# speedup vs baseline: 1.2793x; 1.2793x over previous
"""LocalContrastEnhancement (15x15 box filter mean/var normalization) on 8 trn2 cores.

out = (x - mean) / (sqrt(max(var, 1e-6)) + 1e-6)
mean = box15(x)/225, var = box15(x^2)/225 - mean^2, zero-padded box filter.

Sharding: pure data parallel, 1 image (3,1024,1024) per NeuronCore.

Per-core algorithm (exact at borders via the "pad = -0.5" centering trick):
  x' = x - 0.5 with padding treated as value -0.5 (raw pad zeros in the
  x buffer; the squares stream computes (0-0.5)^2 = 0.25 for pads inline).
  Horizontal 15-window sums are computed by CUSTOM DVE scan ops (the stock
  tensor_tensor_scan routes the scan feedback through out_a and runs at
  half throughput, ~2.2 cyc/elem; a custom Spec scan() lowers to the
  one-cycle CURR_ALU_OUT recurrence, ~1.14 cyc/elem — measured 1229ns vs
  2298ns per [<=128, 1031] scan):
    o1[t] = init0 + sum_{i<=t} (x[i] - x[i-15])          (init0 = 0)
    o2[t] = 3.75 + sum_{i<=t} ((x[i]-.5)^2 - (x[i-15]-.5)^2)
  o1 carries a +7.5 shift vs the centered window sum (init0 = 0 instead of
  -7.5).  After the vertical band matmul with -1 taps this makes
    PD = -(band x o1) = -S1_true     exactly, for ALL stripe variants
  (each band column m has taps(m) = 15 - n(m) taps; the 7.5*taps(m) shift
  equals the old per-row d_scal correction).  So:
    s1sq = (PD + 112.5)^2 = (S1_true - 112.5)^2 = S1~^2   (uniform bias!)
    PD += 225x (fp16 identity matmul)  ->  PD = 225(x - mean)
    P2 = 225*S2~ - s1sq (via -I matmul), rsqrt bias = 843.75*n(m)
    out = PD * rsqrt(P2 + bias)        (plain tensor_tensor mult)

Engine mapping (v3):
  - horizontal 15-box sums: 2 custom DVE scans straight off the raw x
    buffer (the squaring is fused into the scan body: sq(Src-0.5));
  - vertical 15-box: PE band matmuls (fp16, free-dim bound);
  - ACT (one table set): x->fp16 copy, s1sq square, rsqrt;
  - DVE: 2 custom scans + final PD*R tensor_tensor.
"""

import numpy as np
import ml_dtypes

C, H, W = 3, 1024, 1024
NCORES = 8
KS = 15
HALF = 7  # kernel_size // 2
PADL = 15  # left zero pad cols in the row buffer
PADR = 7  # right zero pad cols
BW = PADL + W + PADR  # 1046 row buffer width
SCAN_N = W + HALF  # 1031 scan output length (first 7 are t<0 positions)
MSTR = 114  # interior out-stripe height (128 - 14 halo)
NHALF = 512  # matmul moving free size (one PSUM bank of f32)

_CACHE = {}


def _get_custom_ops():
    """Register (once) and return the two custom DVE scan ops."""
    if "ops" in _CACHE:
        return _CACHE["ops"]
    import concourse.dve_ops as D
    from concourse.dve_uop import DveOpSpec
    from concourse.dve_spec import (
        Spec, Src0, Src1, C0, C1, AluOp, scan, sq, lower, _has_src1,
    )

    def make_op(name, spec):
        for o in D.OPS:
            if o.name == name:
                return o
        row = D._CUSTOM_DVE_ROW_BASE + len(D.OPS)
        shas = {}
        for ver in ("v3", "v4"):
            s = DveOpSpec(
                name=name, opcode=row, uops=lower(spec, ver=ver),
                rd1_en=_has_src1(spec),
            )
            shas[ver] = s.sha(ver)
        op = D.DveOp(name, spec, subdim=False, uops_sha=shas)
        D.OPS.append(op)
        D._SUB_OPCODE_FOR_NAME[name] = row
        D.CUSTOM_DVE_SPECS[name] = spec
        return op

    box = make_op(
        "LCE_BOX_SCAN",
        Spec(
            body=scan(AluOp.ADD, Src0 - Src1, init=C0),
            reference=lambda in0, in1, s0, s1, imm2: s0
            + np.cumsum(in0 - in1, axis=-1),
        ),
    )
    from concourse.dve_spec import C2

    boxsq = make_op(
        "LCE_BOXSQ_SCAN_SC",
        Spec(
            body=scan(AluOp.ADD, sq(Src0 - C1) - sq(Src1 - C1), init=C0) * C2,
            reference=lambda in0, in1, s0, s1, imm2: (
                s0 + np.cumsum((in0 - s1) ** 2 - (in1 - s1) ** 2, axis=-1)
            )
            * imm2,
        ),
    )
    _CACHE["ops"] = (box, boxsq)
    return _CACHE["ops"]


def _stripes():
    """(r_in0, K, r_out0, M, k_ofs) per stripe; k_ofs=7 marks the top stripe
    (its band/id constants are the mid ones shifted up 7 rows)."""
    out = []
    r_out = 0
    while r_out < H:
        m = min(MSTR, H - r_out)
        r_in0 = max(r_out - HALF, 0)
        r_in1 = min(r_out + m - 1 + HALF, H - 1)
        k = r_in1 - r_in0 + 1
        k_ofs = HALF - (r_out - r_in0)
        out.append((r_in0, k, r_out, m, k_ofs))
        r_out += m
    return out


def _const_mats():
    band = np.zeros((128, MSTR), dtype=np.float32)
    iden = np.zeros((128, MSTR), dtype=np.float32)
    for m in range(MSTR):
        band[m : m + KS, m] = 1.0
        iden[m + HALF, m] = 225.0
    band_top = np.zeros_like(band)
    band_top[0:121, :] = band[7:128, :]
    iden_top = np.zeros_like(iden)
    iden_top[0:121, :] = iden[7:128, :]
    # negI for the var fold: out row m subtracts s1sq row m (same partition)
    negi = np.zeros((128, MSTR), dtype=np.float32)
    for m in range(MSTR):
        negi[m, m] = -1.0
    # One shared stationary (-band) serves BOTH the o1 (mean) and the
    # -225-scaled o2 (squares) phase-1 matmuls.
    bands = np.stack([-band, -band_top, negi], axis=1)  # [128, 3, 114] fp16
    idens = np.stack([iden, iden_top], axis=1).astype(np.float16)  # [128, 2, 114]

    # Per-out-row rsqrt bias: vertical windows past the image miss n all-pad
    # rows, each contributing 3.75 to S2~ (so 843.75*n to 225*S2~).  Column
    # 3 holds the uniform +112.5 bias for the s1sq square.
    m_idx = np.arange(128)
    n_top = np.maximum(0, HALF - m_idx).astype(np.float32)
    n_bot = np.maximum(0, m_idx - 104).astype(np.float32)  # bottom stripe M=112
    corr = np.zeros((128, 4), dtype=np.float32)
    corr[:, 0] = 843.75 * n_top
    corr[:, 1] = 843.75 * n_bot
    corr[:, 2] = 0.0  # interior
    corr[:, 3] = 112.5
    return bands.astype(np.float16), idens, corr


def _build_nc():
    import concourse.bass as bass
    import concourse.bacc as bacc
    import concourse.tile as tile
    from concourse import mybir
    import bass_rust as _bass_rust
    from concourse.hw_specs import get_activation_tables

    f32 = mybir.dt.float32
    fp16 = mybir.dt.float16
    Alu = mybir.AluOpType
    Act = mybir.ActivationFunctionType

    BOX, BOXSQ = _get_custom_ops()

    class _LceBacc(bacc.Bacc):
        """Bacc with act-table selection pinned to the one set that holds
        Square+Copy+Abs_reciprocal_sqrt (the default chooser thrashes)."""

        def insert_act_table_loads(self):
            tables = [
                (name, funcs if name == "abs_reciprocal_sqrt_and_small" else set())
                for name, funcs in get_activation_tables(self.m.arch).items()
            ]
            _bass_rust.insert_act_table_loads(self, tables)

    nc = _LceBacc(trn_type="TRN2", target_bir_lowering=False)
    x_d = nc.dram_tensor("x", [C, H, W], f32, kind="ExternalInput")
    bands_d = nc.dram_tensor("bands", [128, 3, MSTR], fp16, kind="ExternalInput")
    iden_d = nc.dram_tensor("iden", [128, 2, MSTR], fp16, kind="ExternalInput")
    corr_d = nc.dram_tensor("corr", [128, 4], f32, kind="ExternalInput")
    y_d = nc.dram_tensor("y", [C, H, W], f32, kind="ExternalOutput")

    stripes = _stripes()

    from contextlib import ExitStack

    with tile.TileContext(nc) as tc, ExitStack() as ctx:
        singles = ctx.enter_context(tc.tile_pool(name="singles", bufs=1))
        io_pool = ctx.enter_context(tc.tile_pool(name="io", bufs=1))
        s1sq_p = ctx.enter_context(tc.tile_pool(name="s1sq", bufs=4))
        r_p = ctx.enter_context(tc.tile_pool(name="rts", bufs=3))
        out_p = ctx.enter_context(tc.tile_pool(name="outb", bufs=4))
        ps_p = ctx.enter_context(tc.tile_pool(name="ps", bufs=2, space="PSUM"))

        bands_t = singles.tile([128, 3, MSTR], fp16)
        iden_t = singles.tile([128, 2, MSTR], fp16)
        corr_t = singles.tile([128, 4], f32)
        nc.sync.dma_start(out=bands_t[:, :, :], in_=bands_d[:, :, :])
        nc.sync.dma_start(out=iden_t[:, :, :], in_=iden_d[:, :, :])
        nc.sync.dma_start(out=corr_t[:, :], in_=corr_d[:, :])

        NBUF = 4
        xb = [io_pool.tile([128, BW], f32, tag=f"xb{i}", name=f"xb{i}") for i in range(NBUF)]
        ob1 = [io_pool.tile([128, SCAN_N], fp16, tag=f"ob1{i}", name=f"ob1{i}") for i in range(NBUF)]
        ob2 = [io_pool.tile([128, SCAN_N], fp16, tag=f"ob2{i}", name=f"ob2{i}") for i in range(NBUF)]
        x16 = [io_pool.tile([128, W], fp16, tag=f"x16{i}", name=f"x16{i}") for i in range(NBUF)]
        for i in range(NBUF):
            nc.vector.memset(xb[i][:, 0:PADL], 0.0)
            nc.vector.memset(xb[i][:, PADL + W : BW], 0.0)

        # ACT hardware instructions carry at most ONE sync wait; warm-up
        # activations make ACT observe the const-DMA queues and DVE memset
        # ticks here so loop activations don't accumulate extra waits.
        warm1 = singles.tile([128, 1], f32)
        warm2 = singles.tile([128, 1], f32)
        warm3 = singles.tile([128, 1], f32)
        warm4 = singles.tile([128, 1], f32)
        nc.scalar.activation(out=warm1[:, :], in_=corr_t[:, 0:1], func=Act.Square)
        nc.scalar.activation(out=warm2[:, :], in_=iden_t[:, 0, 0:1], func=Act.Square)
        nc.scalar.activation(out=warm3[:, :], in_=xb[0][:, 0:1], func=Act.Square)
        nc.scalar.activation(
            out=warm4[:, :], in_=warm3[:, :], func=Act.Abs_reciprocal_sqrt
        )

        it = 0
        for c in range(C):
            for r_in0, K, r_out0, M, k_ofs in stripes:
                i3 = it % NBUF
                it += 1
                xt, o1, o2, xh = xb[i3], ob1[i3], ob2[i3], x16[i3]

                nc.sync.dma_start(
                    out=xt[0:K, PADL : PADL + W],
                    in_=x_d[c, r_in0 : r_in0 + K, :],
                )

                # fp16 copy of x for the (fast) fp16 identity matmul.
                # NOT on GpSimd: its CAST is 4x slower AND contends with the
                # DVE scans for the shared SBUF port (measured 1230->2491ns).
                nc.scalar.activation(
                    out=xh[0:K, :],
                    in_=xt[0:K, PADL : PADL + W],
                    func=Act.Copy,
                )

                # Horizontal sliding 15-sums via the custom DVE scans.
                # o1[t] = 0 + sum_{i<=t} (x[i] - x[i-15])   (pads contribute
                # the exact centering; +7.5-shifted output = the d-fold)
                nc.vector._custom_dve(
                    BOX,
                    out=o1[0:K, 0:SCAN_N],
                    in0=xt[0:K, PADL : PADL + SCAN_N],
                    in1=xt[0:K, 0:SCAN_N],
                    s0=0.0,
                )
                # o2[t] = -225 * (3.75 + sum ((x[i]-.5)^2 - (x[i-15]-.5)^2))
                # (the -225 scale makes the shared -band stationary produce
                # +225*S2~ in the P2 half of the PSUM tile)
                nc.vector._custom_dve(
                    BOXSQ,
                    out=o2[0:K, 0:SCAN_N],
                    in0=xt[0:K, PADL : PADL + SCAN_N],
                    in1=xt[0:K, 0:SCAN_N],
                    s0=3.75,
                    s1=0.5,
                    imm2=-225.0,
                )

                bsel = 1 if k_ofs else 0  # top-stripe band constants at +1
                isel = 1 if k_ofs else 0
                vv = 0 if k_ofs else (1 if r_out0 + M == H else 2)
                p2_bias = corr_t[0:M, vv : vv + 1]
                sq_bias = corr_t[0:M, 3:4]  # +112.5, uniform

                # one 4-bank PSUM tile: cols [0,W) = PD, cols [W,2W) = P2
                pp = ps_p.tile([MSTR, 2 * W], f32)
                # phase 1 (one shared stationary, 4 moving blocks):
                #   PD = -S1_true ; P2 = 225*S2~
                for j0, src in (
                    (0, o1),
                    (NHALF, o1),
                    (W, o2),
                    (W + NHALF, o2),
                ):
                    nc.tensor.matmul(
                        pp[0:M, j0 : j0 + NHALF],
                        bands_t[0:K, bsel, 0:M],
                        src[0:K, HALF + (j0 % W) : HALF + (j0 % W) + NHALF],
                        start=True,
                        stop=False,
                    )
                # s1sq = (S1_true - 112.5)^2 = (PD + 112.5)^2, fp16
                s1sq = s1sq_p.tile([MSTR, W], fp16)
                nc.scalar.activation(
                    out=s1sq[0:M, :],
                    in_=pp[0:M, 0:W],
                    func=Act.Square,
                    bias=sq_bias,
                )
                # phase 2: PD += 225x ; P2 -= s1sq
                for j0 in (0, NHALF):
                    nc.tensor.matmul(
                        pp[0:M, j0 : j0 + NHALF],
                        iden_t[0:K, isel, 0:M],
                        xh[0:K, j0 : j0 + NHALF],
                        start=False,
                        stop=True,
                        skip_group_check=True,
                    )
                for j0 in (0, NHALF):
                    nc.tensor.matmul(
                        pp[0:M, W + j0 : W + j0 + NHALF],
                        bands_t[0:M, 2, 0:M],
                        s1sq[0:M, j0 : j0 + NHALF],
                        start=False,
                        stop=True,
                    )
                # R = rsqrt(var') in one ACT op (probed: 4.4e-5 max rel
                # err); var' = P2 + corr folded into the activation bias.
                rts = r_p.tile([MSTR, W], f32)
                nc.scalar.activation(
                    out=rts[0:M, :],
                    in_=pp[0:M, W : 2 * W],
                    func=Act.Abs_reciprocal_sqrt,
                    bias=p2_bias,
                )
                # out = PD * R
                outb = out_p.tile([MSTR, W], f32)
                nc.vector.tensor_tensor(
                    out=outb[0:M, :],
                    in0=pp[0:M, 0:W],
                    in1=rts[0:M, :],
                    op=Alu.mult,
                )
                nc.sync.dma_start(
                    out=y_d[c, r_out0 : r_out0 + M, :], in_=outb[0:M, :]
                )

    nc.finalize()
    return nc


def _get_nc():
    if "nc" not in _CACHE:
        _CACHE["nc"] = _build_nc()
    return _CACHE["nc"]


def kernel(x: np.ndarray, _trace: bool = False, _tmpdir=None) -> np.ndarray:
    from concourse.bass_utils import run_bass_kernel_spmd

    assert x.shape == (NCORES, C, H, W), x.shape
    nc = _get_nc()
    bands, iden, corr = _const_mats()
    in_maps = [
        {
            "x": np.ascontiguousarray(x[i]).astype(np.float32, copy=False),
            "bands": bands,
            "iden": iden,
            "corr": corr,
        }
        for i in range(NCORES)
    ]
    res = run_bass_kernel_spmd(
        nc,
        in_maps,
        core_ids=list(range(NCORES)),
        trace=_trace,
        tmpdir=_tmpdir,
    )
    _CACHE["last_results"] = res
    out = np.stack([r["y"] for r in res.results], axis=0)
    return out


if __name__ == "__main__":
    rng = np.random.default_rng(0)
    x = rng.random((NCORES, C, H, W), dtype=np.float32)
    y = kernel(x)
    print(y.shape, y.dtype, float(np.abs(y).mean()))


# revision 20
# speedup vs baseline: 1.3606x; 1.0635x over previous
"""LocalContrastEnhancement (15x15 box filter mean/var normalization) on 8 trn2 cores.

out = (x - mean) / (sqrt(max(var, 1e-6)) + 1e-6)
mean = box15(x)/225, var = box15(x^2)/225 - mean^2, zero-padded box filter.

Sharding: pure data parallel, 1 image (3,1024,1024) per NeuronCore.

Per-core algorithm (exact at borders via the "pad = -0.5" centering trick):
  x' = x - 0.5 with padding treated as value -0.5 (raw pad zeros in the
  x buffer; the squares stream computes (0-0.5)^2 = 0.25 for pads inline).
  Horizontal 15-window sums are computed by CUSTOM DVE scan ops (the stock
  tensor_tensor_scan routes the scan feedback through out_a and runs at
  half throughput, ~2.2 cyc/elem; a custom Spec scan() lowers to the
  one-cycle CURR_ALU_OUT recurrence, ~1.14 cyc/elem — measured 1229ns vs
  2298ns per [<=128, 1031] scan):
    o1[t] = init0 + sum_{i<=t} (x[i] - x[i-15])          (init0 = 0)
    o2[t] = 3.75 + sum_{i<=t} ((x[i]-.5)^2 - (x[i-15]-.5)^2)
  o1 carries a +7.5 shift vs the centered window sum (init0 = 0 instead of
  -7.5).  After the vertical band matmul with -1 taps this makes
    PD = -(band x o1) = -S1_true     exactly, for ALL stripe variants
  (each band column m has taps(m) = 15 - n(m) taps; the 7.5*taps(m) shift
  equals the old per-row d_scal correction).  So:
    s1sq = (PD + 112.5)^2 = (S1_true - 112.5)^2 = S1~^2   (uniform bias!)
    PD += 225x (fp16 identity matmul)  ->  PD = 225(x - mean)
    P2 = 225*S2~ - s1sq (via -I matmul), rsqrt bias = 843.75*n(m)
    out = PD * rsqrt(P2 + bias)        (plain tensor_tensor mult)

Engine mapping (v3):
  - horizontal 15-box sums: 2 custom DVE scans straight off the raw x
    buffer (the squaring is fused into the scan body: sq(Src-0.5));
  - vertical 15-box: PE band matmuls (fp16, free-dim bound);
  - ACT (one table set): x->fp16 copy, s1sq square, rsqrt;
  - DVE: 2 custom scans + final PD*R tensor_tensor.
"""

import numpy as np
import ml_dtypes

C, H, W = 3, 1024, 1024
NCORES = 8
KS = 15
HALF = 7  # kernel_size // 2
PADL = 15  # left zero pad cols in the row buffer
PADR = 7  # right zero pad cols
BW = PADL + W + PADR  # 1046 row buffer width
SCAN_N = W + HALF  # 1031 scan output length (first 7 are t<0 positions)
MSTR = 114  # interior out-stripe height (128 - 14 halo)
NHALF = 512  # matmul moving free size (one PSUM bank of f32)

_CACHE = {}


def _get_custom_ops():
    """Register (once) and return the two custom DVE scan ops."""
    if "ops" in _CACHE:
        return _CACHE["ops"]
    import concourse.dve_ops as D
    from concourse.dve_uop import DveOpSpec
    from concourse.dve_spec import (
        Spec, Src0, Src1, C0, C1, AluOp, scan, sq, lower, _has_src1,
    )

    def make_op(name, spec):
        for o in D.OPS:
            if o.name == name:
                return o
        row = D._CUSTOM_DVE_ROW_BASE + len(D.OPS)
        shas = {}
        for ver in ("v3", "v4"):
            s = DveOpSpec(
                name=name, opcode=row, uops=lower(spec, ver=ver),
                rd1_en=_has_src1(spec),
            )
            shas[ver] = s.sha(ver)
        op = D.DveOp(name, spec, subdim=False, uops_sha=shas)
        D.OPS.append(op)
        D._SUB_OPCODE_FOR_NAME[name] = row
        D.CUSTOM_DVE_SPECS[name] = spec
        return op

    box = make_op(
        "LCE_BOX_SCAN",
        Spec(
            body=scan(AluOp.ADD, Src0 - Src1, init=C0),
            reference=lambda in0, in1, s0, s1, imm2: s0
            + np.cumsum(in0 - in1, axis=-1),
        ),
    )
    from concourse.dve_spec import C2

    boxsq = make_op(
        "LCE_BOXSQ_SCAN_SC",
        Spec(
            body=scan(AluOp.ADD, sq(Src0 - C1) - sq(Src1 - C1), init=C0) * C2,
            reference=lambda in0, in1, s0, s1, imm2: (
                s0 + np.cumsum((in0 - s1) ** 2 - (in1 - s1) ** 2, axis=-1)
            )
            * imm2,
        ),
    )
    _CACHE["ops"] = (box, boxsq)
    return _CACHE["ops"]


def _stripes():
    """(r_in0, K, r_out0, M, k_ofs) per stripe; k_ofs=7 marks the top stripe
    (its band/id constants are the mid ones shifted up 7 rows)."""
    out = []
    r_out = 0
    while r_out < H:
        m = min(MSTR, H - r_out)
        r_in0 = max(r_out - HALF, 0)
        r_in1 = min(r_out + m - 1 + HALF, H - 1)
        k = r_in1 - r_in0 + 1
        k_ofs = HALF - (r_out - r_in0)
        out.append((r_in0, k, r_out, m, k_ofs))
        r_out += m
    return out


def _const_mats():
    band = np.zeros((128, MSTR), dtype=np.float32)
    iden = np.zeros((128, MSTR), dtype=np.float32)
    for m in range(MSTR):
        band[m : m + KS, m] = 1.0
        iden[m + HALF, m] = 225.0
    band_top = np.zeros_like(band)
    band_top[0:121, :] = band[7:128, :]
    iden_top = np.zeros_like(iden)
    iden_top[0:121, :] = iden[7:128, :]
    # negI for the var fold: out row m subtracts s1sq row m (same partition)
    negi = np.zeros((128, MSTR), dtype=np.float32)
    for m in range(MSTR):
        negi[m, m] = -1.0
    # One shared stationary (-band) serves BOTH the o1 (mean) and the
    # -225-scaled o2 (squares) phase-1 matmuls.
    bands = np.stack([-band, -band_top, negi], axis=1)  # [128, 3, 114] fp16
    # idens stay f32: the identity matmul runs in float32r mode straight off
    # the raw f32 x buffer (no fp16 copy pass needed; f32r moving >=256 cols
    # runs at full 1 cyc/row on the PE)
    idens = np.stack([iden, iden_top], axis=1)  # [128, 2, 114] f32

    # Per-out-row rsqrt bias: vertical windows past the image miss n all-pad
    # rows, each contributing 3.75 to S2~ (so 843.75*n to 225*S2~).  Column
    # 3 holds the uniform +112.5 bias for the s1sq square.
    m_idx = np.arange(128)
    n_top = np.maximum(0, HALF - m_idx).astype(np.float32)
    n_bot = np.maximum(0, m_idx - 104).astype(np.float32)  # bottom stripe M=112
    corr = np.zeros((128, 4), dtype=np.float32)
    corr[:, 0] = 843.75 * n_top
    corr[:, 1] = 843.75 * n_bot
    corr[:, 2] = 0.0  # interior
    corr[:, 3] = 112.5
    return bands.astype(np.float16), idens, corr


def _build_nc():
    import concourse.bass as bass
    import concourse.bacc as bacc
    import concourse.tile as tile
    from concourse import mybir
    import bass_rust as _bass_rust
    from concourse.hw_specs import get_activation_tables

    f32 = mybir.dt.float32
    f32r = mybir.dt.float32r
    fp16 = mybir.dt.float16
    Alu = mybir.AluOpType
    Act = mybir.ActivationFunctionType

    BOX, BOXSQ = _get_custom_ops()

    class _LceBacc(bacc.Bacc):
        """Bacc with act-table selection pinned to the one set that holds
        Square+Copy+Abs_reciprocal_sqrt (the default chooser thrashes)."""

        def insert_act_table_loads(self):
            tables = [
                (name, funcs if name == "abs_reciprocal_sqrt_and_small" else set())
                for name, funcs in get_activation_tables(self.m.arch).items()
            ]
            _bass_rust.insert_act_table_loads(self, tables)

    nc = _LceBacc(trn_type="TRN2", target_bir_lowering=False)
    x_d = nc.dram_tensor("x", [C, H, W], f32, kind="ExternalInput")
    bands_d = nc.dram_tensor("bands", [128, 3, MSTR], fp16, kind="ExternalInput")
    # idens feed the f32r identity matmul; external dtype must stay f32
    # (float32r has no jax mapping) so APs are bitcast at the use sites.
    iden_d = nc.dram_tensor("iden", [128, 2, MSTR], f32, kind="ExternalInput")
    corr_d = nc.dram_tensor("corr", [128, 4], f32, kind="ExternalInput")
    y_d = nc.dram_tensor("y", [C, H, W], f32, kind="ExternalOutput")

    stripes = _stripes()

    from contextlib import ExitStack

    with tile.TileContext(nc) as tc, ExitStack() as ctx:
        singles = ctx.enter_context(tc.tile_pool(name="singles", bufs=1))
        io_pool = ctx.enter_context(tc.tile_pool(name="io", bufs=1))
        s1sq_p = ctx.enter_context(tc.tile_pool(name="s1sq", bufs=4))
        r_p = ctx.enter_context(tc.tile_pool(name="rts", bufs=3))
        out_p = ctx.enter_context(tc.tile_pool(name="outb", bufs=4))
        psd_p = ctx.enter_context(tc.tile_pool(name="psd", bufs=2, space="PSUM"))
        ps2_p = ctx.enter_context(tc.tile_pool(name="ps2", bufs=2, space="PSUM"))

        bands_t = singles.tile([128, 3, MSTR], fp16)
        iden_t = singles.tile([128, 2, MSTR], f32)
        corr_t = singles.tile([128, 4], f32)
        nc.sync.dma_start(out=bands_t[:, :, :], in_=bands_d[:, :, :])
        nc.sync.dma_start(
            out=iden_t[:, :, :].bitcast(f32r), in_=iden_d[:, :, :].bitcast(f32r)
        )
        nc.sync.dma_start(out=corr_t[:, :], in_=corr_d[:, :])

        NBUF = 6
        xb = [io_pool.tile([128, BW], f32, tag=f"xb{i}", name=f"xb{i}") for i in range(NBUF)]
        ob1 = [io_pool.tile([128, SCAN_N], fp16, tag=f"ob1{i}", name=f"ob1{i}") for i in range(NBUF)]
        ob2 = [io_pool.tile([128, SCAN_N], fp16, tag=f"ob2{i}", name=f"ob2{i}") for i in range(NBUF)]
        for i in range(NBUF):
            nc.vector.memset(xb[i][:, 0:PADL], 0.0)
            nc.vector.memset(xb[i][:, PADL + W : BW], 0.0)

        # ACT hardware instructions carry at most ONE sync wait; warm-up
        # activations make ACT observe the const-DMA queues and DVE memset
        # ticks here so loop activations don't accumulate extra waits.
        warm1 = singles.tile([128, 1], f32)
        warm2 = singles.tile([128, 1], f32)
        warm3 = singles.tile([128, 1], f32)
        warm4 = singles.tile([128, 1], f32)
        nc.scalar.activation(out=warm1[:, :], in_=corr_t[:, 0:1], func=Act.Square)
        nc.scalar.activation(out=warm2[:, :], in_=iden_t[:, 0, 0:1], func=Act.Square)
        nc.scalar.activation(out=warm3[:, :], in_=xb[0][:, 0:1], func=Act.Square)
        nc.scalar.activation(
            out=warm4[:, :], in_=warm3[:, :], func=Act.Abs_reciprocal_sqrt
        )

        it = 0
        for c in range(C):
            for r_in0, K, r_out0, M, k_ofs in stripes:
                i3 = it % NBUF
                it += 1
                xt, o1, o2 = xb[i3], ob1[i3], ob2[i3]

                # DMA typed f32r so the BIR verifier accepts the f32r
                # identity matmul reading this buffer (raw bits identical;
                # the scans read the same tile through its f32 APs).
                nc.sync.dma_start(
                    out=xt[0:K, PADL : PADL + W].bitcast(f32r),
                    in_=x_d[c, r_in0 : r_in0 + K, :].bitcast(f32r),
                )

                # Horizontal sliding 15-sums via the custom DVE scans.
                # o1[t] = 0 + sum_{i<=t} (x[i] - x[i-15])   (pads contribute
                # the exact centering; +7.5-shifted output = the d-fold)
                nc.vector._custom_dve(
                    BOX,
                    out=o1[0:K, 0:SCAN_N],
                    in0=xt[0:K, PADL : PADL + SCAN_N],
                    in1=xt[0:K, 0:SCAN_N],
                    s0=0.0,
                )
                # o2[t] = -225 * (3.75 + sum ((x[i]-.5)^2 - (x[i-15]-.5)^2))
                # (the -225 scale makes the shared -band stationary produce
                # +225*S2~ in the P2 half of the PSUM tile)
                nc.vector._custom_dve(
                    BOXSQ,
                    out=o2[0:K, 0:SCAN_N],
                    in0=xt[0:K, PADL : PADL + SCAN_N],
                    in1=xt[0:K, 0:SCAN_N],
                    s0=3.75,
                    s1=0.5,
                    imm2=-225.0,
                )

                bsel = 1 if k_ofs else 0  # top-stripe band constants at +1
                isel = 1 if k_ofs else 0
                vv = 0 if k_ofs else (1 if r_out0 + M == H else 2)
                p2_bias = corr_t[0:M, vv : vv + 1]
                sq_bias = corr_t[0:M, 3:4]  # +112.5, uniform

                pd = psd_p.tile([MSTR, W], f32)
                p2 = ps2_p.tile([MSTR, W], f32)
                # phase 1 (one shared stationary, 4 moving blocks):
                #   PD = -S1_true ; P2 = 225*S2~
                for j0 in (0, NHALF):
                    nc.tensor.matmul(
                        pd[0:M, j0 : j0 + NHALF],
                        bands_t[0:K, bsel, 0:M],
                        o1[0:K, HALF + j0 : HALF + j0 + NHALF],
                        start=True,
                        stop=False,
                    )
                for j0 in (0, NHALF):
                    nc.tensor.matmul(
                        p2[0:M, j0 : j0 + NHALF],
                        bands_t[0:K, bsel, 0:M],
                        o2[0:K, HALF + j0 : HALF + j0 + NHALF],
                        start=True,
                        stop=False,
                    )
                # s1sq = (S1_true - 112.5)^2 = (PD + 112.5)^2, fp16
                s1sq = s1sq_p.tile([MSTR, W], fp16)
                nc.scalar.activation(
                    out=s1sq[0:M, :],
                    in_=pd[0:M, :],
                    func=Act.Square,
                    bias=sq_bias,
                )
                # phase 2: PD += 225x (f32r identity matmul straight off the
                # raw f32 x buffer — no fp16 copy pass) ; P2 -= s1sq
                for j0 in (0, NHALF):
                    nc.tensor.matmul(
                        pd[0:M, j0 : j0 + NHALF],
                        iden_t[0:K, isel, 0:M].bitcast(f32r),
                        xt[0:K, PADL + j0 : PADL + j0 + NHALF].bitcast(f32r),
                        start=False,
                        stop=True,
                        skip_group_check=True,
                    )
                for j0 in (0, NHALF):
                    nc.tensor.matmul(
                        p2[0:M, j0 : j0 + NHALF],
                        bands_t[0:M, 2, 0:M],
                        s1sq[0:M, j0 : j0 + NHALF],
                        start=False,
                        stop=True,
                    )
                # R = rsqrt(var') in one ACT op (probed: 4.4e-5 max rel
                # err); var' = P2 + corr folded into the activation bias.
                rts = r_p.tile([MSTR, W], f32)
                nc.scalar.activation(
                    out=rts[0:M, :],
                    in_=p2[0:M, :],
                    func=Act.Abs_reciprocal_sqrt,
                    bias=p2_bias,
                )
                # out = PD * R
                outb = out_p.tile([MSTR, W], f32)
                nc.vector.tensor_tensor(
                    out=outb[0:M, :],
                    in0=pd[0:M, :],
                    in1=rts[0:M, :],
                    op=Alu.mult,
                )
                nc.sync.dma_start(
                    out=y_d[c, r_out0 : r_out0 + M, :], in_=outb[0:M, :]
                )

    nc.finalize()
    return nc


def _get_nc():
    if "nc" not in _CACHE:
        _CACHE["nc"] = _build_nc()
    return _CACHE["nc"]


def kernel(x: np.ndarray, _trace: bool = False, _tmpdir=None) -> np.ndarray:
    from concourse.bass_utils import run_bass_kernel_spmd

    assert x.shape == (NCORES, C, H, W), x.shape
    nc = _get_nc()
    bands, iden, corr = _const_mats()
    in_maps = [
        {
            "x": np.ascontiguousarray(x[i]).astype(np.float32, copy=False),
            "bands": bands,
            "iden": iden,
            "corr": corr,
        }
        for i in range(NCORES)
    ]
    res = run_bass_kernel_spmd(
        nc,
        in_maps,
        core_ids=list(range(NCORES)),
        trace=_trace,
        tmpdir=_tmpdir,
    )
    _CACHE["last_results"] = res
    out = np.stack([r["y"] for r in res.results], axis=0)
    return out


if __name__ == "__main__":
    rng = np.random.default_rng(0)
    x = rng.random((NCORES, C, H, W), dtype=np.float32)
    y = kernel(x)
    print(y.shape, y.dtype, float(np.abs(y).mean()))
